# revision 8
# baseline (speedup 1.0000x reference)
"""Multi-head attention (B=4, S=2048, D=1024, H=16) on 8 trn2 NeuronCores.

Sharding: core c handles batch b = c//2, head-group g = c%2 (8 of 16 heads).
Each core computes q/k/v projections for its head group, attention for its
8 heads, and a partial output projection (contribution of its head group).
Host sums the two partials per batch and adds the output bias.

Device layout (all matmul operands bf16, f32 accumulation):
  - scores are computed TRANSPOSED: scoresT[k, q] = khT.T @ qhT per 128-k-chunk,
    so softmax exp runs on ACT with q on the free axis and the PV matmul
    (ctxT[d, q] = vh.T @ expT) needs no transposes.
  - softmax denominators come free from a ones-column appended to the PV
    stationary operand (row 64 of the PV psum accumulates sum_k exp).
  - normalization: reciprocal on DVE, partition-broadcast via a tiny
    ones-outer-product matmul on PE, multiply on DVE during psum->sbuf copyback.
"""
import numpy as np
import ml_dtypes
from contextlib import ExitStack

import concourse.bacc as bacc
import concourse.tile as tile
from concourse import mybir
from concourse.bass_utils import run_bass_kernel_spmd

P = 128
S = 2048          # sequence length
D = 1024          # embed dim
HC = 512          # local head columns (8 heads x 64)
NH = 8            # local heads
HD = 64           # head dim
DC = 8            # d chunks (D / 128)
LC = 4            # local-hidden chunks (HC / 128)
KC = 16           # k-position chunks (S / 128)
NSC = 4           # seq chunks of 512
QB = 1024         # q block
SCALE = 0.125     # 1/sqrt(64)

BF16 = mybir.dt.bfloat16
F32 = mybir.dt.float32
EXP = mybir.ActivationFunctionType.Exp

_NC_CACHE = None


def _build(loop_n=None):
    nc = bacc.Bacc()

    xq_d = nc.declare_dram_parameter("xq", [P, DC, S], BF16, isOutput=False)
    xk_d = nc.declare_dram_parameter("xk", [P, DC, S], BF16, isOutput=False)
    xv_d = nc.declare_dram_parameter("xv", [KC, P, DC, P], BF16, isOutput=False)
    wq_d = nc.declare_dram_parameter("wq", [P, DC, HC], BF16, isOutput=False)
    wk_d = nc.declare_dram_parameter("wk", [P, DC, HC], BF16, isOutput=False)
    wv_d = nc.declare_dram_parameter("wv", [P, DC, HC], BF16, isOutput=False)
    wo_d = nc.declare_dram_parameter("wo", [P, LC, D], BF16, isOutput=False)
    bq_d = nc.declare_dram_parameter("bq", [1, HC], BF16, isOutput=False)
    bk_d = nc.declare_dram_parameter("bk", [1, HC], BF16, isOutput=False)
    bv_d = nc.declare_dram_parameter("bv", [1, HC], BF16, isOutput=False)
    out_d = nc.declare_dram_parameter("out", [KC, P, D], F32, isOutput=True)

    with ExitStack() as ctx:
        tc = ctx.enter_context(tile.TileContext(nc))
        pers = ctx.enter_context(tc.tile_pool(name="pers", bufs=1))
        xin = ctx.enter_context(tc.tile_pool(name="xin", bufs=2))
        epool = ctx.enter_context(tc.tile_pool(name="epool", bufs=4))
        rpool = ctx.enter_context(tc.tile_pool(name="rpool", bufs=2))
        pscore = ctx.enter_context(tc.tile_pool(name="pscore", bufs=2, space="PSUM"))
        pctx = ctx.enter_context(tc.tile_pool(name="pctx", bufs=1, space="PSUM"))
        paux = ctx.enter_context(tc.tile_pool(name="paux", bufs=2, space="PSUM"))

        # ---- persistent SBUF tensors ----
        wq = pers.tile([P, DC, HC], BF16, name="wq_t")
        wk = pers.tile([P, DC, HC], BF16, name="wk_t")
        wv = pers.tile([P, DC, HC], BF16, name="wv_t")
        wo = pers.tile([P, LC, D], BF16, name="wo_t")
        bq = pers.tile([1, HC], BF16, name="bq_t")
        bk = pers.tile([1, HC], BF16, name="bk_t")
        bv = pers.tile([1, HC], BF16, name="bv_t")
        ones = pers.tile([1, 512], BF16, name="ones_t")
        qh = pers.tile([P, LC, S], BF16, name="qh_t")     # qhT: [head-col, seq]
        kh = pers.tile([P, LC, S], BF16, name="kh_t")     # khT
        vh = pers.tile([P, KC, NH, HD + 1], BF16, name="vh_t")  # [k-seq, ., head, d|1]
        ctxT = pers.tile([P, LC, S], BF16, name="ctx_t")  # [local-hidden, seq]

        nc.sync.dma_start(wq[:], wq_d[:])
        nc.sync.dma_start(wk[:], wk_d[:])
        nc.sync.dma_start(wv[:], wv_d[:])
        nc.sync.dma_start(wo[:], wo_d[:])
        nc.sync.dma_start(bq[:], bq_d[:])
        nc.sync.dma_start(bk[:], bk_d[:])
        nc.sync.dma_start(bv[:], bv_d[:])
        nc.vector.memset(ones[:], 1.0)
        nc.vector.memset(vh[:, :, :, HD:HD + 1], 1.0)  # PV ones-column

        loop_cm = tc.For_i(0, loop_n, 1) if loop_n else None
        if loop_cm:
            loop_cm.__enter__()
        xq = xin.tile([P, DC, S], BF16, name="xq_t", tag="x")
        for dc in range(DC):
            nc.sync.dma_start(xq[:, dc, :], xq_d[:, dc, :])
        xk = xin.tile([P, DC, S], BF16, name="xk_t", tag="x")
        for dc in range(DC):
            nc.sync.dma_start(xk[:, dc, :], xk_d[:, dc, :])

        def proj_qk(dst, x_t, w_t, b_t, c):
            """dst[:, c, :] = (W_g @ x.T + b) for head-col chunk c, all seq."""
            for sc in range(NSC):
                pt = paux.tile([P, 512], F32, name=f"pp_{c}_{sc}", tag="aux")
                for dc in range(DC):
                    nc.tensor.matmul(
                        pt[:], lhsT=w_t[:, dc, c * P:(c + 1) * P],
                        rhs=x_t[:, dc, sc * 512:(sc + 1) * 512],
                        start=(dc == 0), stop=False)
                nc.tensor.matmul(
                    pt[:], lhsT=b_t[0:1, c * P:(c + 1) * P], rhs=ones[0:1, :],
                    start=False, stop=True)
                nc.vector.tensor_copy(dst[:, c, sc * 512:(sc + 1) * 512], pt[:])

        def proj_v(sc):
            """vh[:, sc, :, 0:64] = (xv @ Wv.T + bv) rows sc*128..+128."""
            xvc = xin.tile([P, DC, P], BF16, name=f"xvc_{sc}", tag="xvc", bufs=4)
            nc.sync.dma_start(xvc[:], xv_d[sc])
            pt = paux.tile([P, 512], F32, name=f"pv_{sc}", tag="aux")
            for dc in range(DC):
                nc.tensor.matmul(
                    pt[:], lhsT=xvc[:, dc, :], rhs=wv[:, dc, :],
                    start=(dc == 0), stop=False)
            nc.tensor.matmul(
                pt[:], lhsT=ones[0:1, 0:P], rhs=bv[0:1, :], start=False, stop=True)
            nc.vector.tensor_copy(
                vh[:, sc, :, 0:HD], pt[:].rearrange("p (h d) -> p h d", h=NH))

        def attn_block(h, qb, kcs):
            """QK + exp + PV for head h, q-block qb, k-chunks kcs."""
            c, po = h // 2, (h % 2) * HD
            q0 = qb * QB
            ctx_t = attn_block.ctx
            for kc in kcs:
                st = pscore.tile([P, QB], F32, name=f"st_{h}_{qb}_{kc}", tag="scores")
                for hf in range(2):
                    nc.tensor.matmul(
                        st[:, hf * 512:(hf + 1) * 512],
                        lhsT=kh[po:po + HD, c, kc * P:(kc + 1) * P],
                        rhs=qh[po:po + HD, c, q0 + hf * 512:q0 + (hf + 1) * 512],
                        start=True, stop=True, tile_position=(po, 0))
                ex = epool.tile([P, QB], BF16, name=f"ex_{h}_{qb}_{kc}", tag="exp", bufs=3)
                nc.scalar.activation(ex[:], st[:], EXP, scale=SCALE)
                for hf in range(2):
                    nc.tensor.matmul(
                        ctx_t[:, hf * 512:(hf + 1) * 512],
                        lhsT=vh[:, kc, h, :],
                        rhs=ex[:, hf * 512:(hf + 1) * 512],
                        start=(kc == 0), stop=(kc == KC - 1))

        def attn_norm(h, qb):
            """Normalize ctx psum by softmax denominators into ctxT sbuf."""
            c, po = h // 2, (h % 2) * HD
            q0 = qb * QB
            ctx_t = attn_block.ctx
            dn = rpool.tile([1, QB], F32, name=f"dn_{h}_{qb}", tag="den")
            nc.vector.tensor_copy(dn[:], ctx_t[HD:HD + 1, :])
            rf = rpool.tile([1, QB], F32, name=f"rf_{h}_{qb}", tag="recf")
            nc.vector.reciprocal_approx_fast(rf[:], dn[:])
            rb = rpool.tile([1, QB], BF16, name=f"rb_{h}_{qb}", tag="recb")
            nc.vector.tensor_copy(rb[:], rf[:])
            for hf in range(2):
                bc = paux.tile([P, 512], F32, name=f"bc_{h}_{qb}_{hf}", tag="aux")
                nc.tensor.matmul(
                    bc[0:HD, :], lhsT=ones[0:1, 0:HD],
                    rhs=rb[0:1, hf * 512:(hf + 1) * 512], start=True, stop=True)
                bcs = rpool.tile([HD, 512], F32, name=f"bcs_{h}_{qb}_{hf}", tag="bcs")
                nc.vector.tensor_copy(bcs[:], bc[0:HD, :])
                nc.vector.tensor_mul(
                    ctxT[po:po + HD, c, q0 + hf * 512:q0 + (hf + 1) * 512],
                    ctx_t[0:HD, hf * 512:(hf + 1) * 512], bcs[:])

        def attn_head(h, weave=None):
            for qb in range(2):
                attn_block.ctx = pctx.tile(
                    [HD + 1, QB], F32, name=f"ctx_{h}_{qb}", tag="ctx")
                for k0 in range(0, KC, 4):
                    attn_block(h, qb, range(k0, k0 + 4))
                    if weave:
                        for fn in weave.pop(0) if weave else []:
                            fn()
                attn_norm(h, qb)

        def outproj():
            for sc in range(KC):
                for dcol in range(2):
                    ot = paux.tile([P, 512], F32, name=f"op_{sc}_{dcol}", tag="aux")
                    for lc in range(LC):
                        nc.tensor.matmul(
                            ot[:], lhsT=ctxT[:, lc, sc * P:(sc + 1) * P],
                            rhs=wo[:, lc, dcol * 512:(dcol + 1) * 512],
                            start=(lc == 0), stop=(lc == LC - 1))
                    og = epool.tile([P, 512], F32, name=f"og_{sc}_{dcol}", tag="ostage", bufs=2)
                    nc.scalar.copy(og[:], ot[:])
                    nc.sync.dma_start(out_d[sc, :, dcol * 512:(dcol + 1) * 512], og[:])

        # ---- emission order == scheduling priority ----
        proj_qk(qh, xq, wq, bq, 0)
        proj_qk(kh, xk, wk, bk, 0)
        for sc in range(4):
            proj_v(sc)
        # head 0 with just-in-time vh projection woven between kc blocks
        weave = [[lambda s=s: proj_v(s) for s in range(4, 8)],
                 [lambda s=s: proj_v(s) for s in range(8, 12)],
                 [lambda s=s: proj_v(s) for s in range(12, 16)],
                 []]
        attn_head(0, weave=weave)
        attn_head(1)
        for c in range(1, LC):
            proj_qk(qh, xq, wq, bq, c)
            proj_qk(kh, xk, wk, bk, c)
            attn_head(2 * c)
            attn_head(2 * c + 1)
        outproj()
        if loop_cm:
            loop_cm.__exit__(None, None, None)

    nc.finalize()
    return nc


def _get_nc():
    global _NC_CACHE
    if _NC_CACHE is None:
        _NC_CACHE = _build()
    return _NC_CACHE


def _to_dev_x(x):
    # [S, D] f32 -> [P, DC, S] bf16 with [p, dc, s] = x[s, dc*128+p]
    return np.ascontiguousarray(
        x.T.reshape(DC, P, S).transpose(1, 0, 2)).astype(ml_dtypes.bfloat16)


def _to_dev_xv(x):
    # [S, D] f32 -> [KC, P, DC, P] bf16 with [sc, p, dc, s'] = x[sc*128+s', dc*128+p]
    return np.ascontiguousarray(
        x.reshape(KC, P, DC, P).transpose(0, 3, 2, 1)).astype(ml_dtypes.bfloat16)


def _to_dev_w(w, g):
    # Wg = W[g*512:(g+1)*512, :]; dev[p, dc, j] = Wg[j, dc*128+p]
    wg = w[g * HC:(g + 1) * HC, :]
    return np.ascontiguousarray(
        wg.T.reshape(DC, P, HC).transpose(1, 0, 2)).astype(ml_dtypes.bfloat16)


def _to_dev_wo(w, g):
    # Wog = W[:, g*512:(g+1)*512]; dev[p, lc, j] = Wog[j, lc*128+p].T ...
    wog = w[:, g * HC:(g + 1) * HC]  # [D, HC]
    return np.ascontiguousarray(
        wog.T.reshape(LC, P, D).transpose(1, 0, 2)).astype(ml_dtypes.bfloat16)


def kernel(q, k, v, Wq, bq, Wk, bk, Wv, bv, Wo, bo):
    q, k, v = (np.asarray(t, np.float32) for t in (q, k, v))
    Wq, Wk, Wv, Wo = (np.asarray(t, np.float32) for t in (Wq, Wk, Wv, Wo))
    bq, bk, bv, bo = (np.asarray(t, np.float32) for t in (bq, bk, bv, bo))
    nc = _get_nc()

    B = q.shape[0]
    xqs = [_to_dev_x(q[b]) for b in range(B)]
    xks = [_to_dev_x(k[b]) for b in range(B)]
    xvs = [_to_dev_xv(v[b]) for b in range(B)]
    wdev = {}
    for g in range(2):
        wdev[g] = dict(
            wq=_to_dev_w(Wq, g), wk=_to_dev_w(Wk, g), wv=_to_dev_w(Wv, g),
            wo=_to_dev_wo(Wo, g),
            bq=bq[g * HC:(g + 1) * HC].reshape(1, HC).astype(ml_dtypes.bfloat16),
            bk=bk[g * HC:(g + 1) * HC].reshape(1, HC).astype(ml_dtypes.bfloat16),
            bv=bv[g * HC:(g + 1) * HC].reshape(1, HC).astype(ml_dtypes.bfloat16),
        )
    in_maps = []
    for c in range(8):
        b, g = c // 2, c % 2
        in_maps.append(dict(xq=xqs[b], xk=xks[b], xv=xvs[b], **wdev[g]))

    res = run_bass_kernel_spmd(nc, in_maps, core_ids=list(range(8)))

    out = np.empty((B, S, D), np.float32)
    for b in range(B):
        p0 = res.results[2 * b]["out"].reshape(S, D)
        p1 = res.results[2 * b + 1]["out"].reshape(S, D)
        out[b] = p0 + p1 + bo
    return out


# revision 9
# speedup vs baseline: 5.4182x; 5.4182x over previous
"""Multi-head attention (B=4, S=2048, D=1024, H=16) on 8 trn2 NeuronCores.

Sharding: core c handles batch b = c//2, head-group g = c%2 (8 of 16 heads).
Each core computes q/k/v projections for its head group, attention for its
8 heads, and a partial output projection (contribution of its head group).
Host sums the two partials per batch and adds the output bias.

Device layout (all matmul operands bf16, f32 accumulation):
  - scores are computed TRANSPOSED: scoresT[k, q] = khT.T @ qhT per 128-k-chunk,
    so softmax exp runs on ACT with q on the free axis and the PV matmul
    (ctxT[d, q] = vh.T @ expT) needs no transposes.
  - softmax denominators come free from a ones-column appended to the PV
    stationary operand (row 64 of the PV psum accumulates sum_k exp).
  - normalization: reciprocal on DVE, partition-broadcast via a tiny
    ones-outer-product matmul on PE, multiply on DVE during psum->sbuf copyback.
"""
import numpy as np
import ml_dtypes
from contextlib import ExitStack

import concourse.bacc as bacc
import concourse.tile as tile
from concourse import mybir
from concourse.bass_utils import run_bass_kernel_spmd

P = 128
S = 2048          # sequence length
D = 1024          # embed dim
HC = 512          # local head columns (8 heads x 64)
NH = 8            # local heads
HD = 64           # head dim
DC = 8            # d chunks (D / 128)
LC = 4            # local-hidden chunks (HC / 128)
KC = 16           # k-position chunks (S / 128)
NSC = 4           # seq chunks of 512
QB = 1024         # q block
SCALE = 0.125     # 1/sqrt(64)

BF16 = mybir.dt.bfloat16
F32 = mybir.dt.float32
EXP = mybir.ActivationFunctionType.Exp

_NC_CACHE = None


def _build(loop_n=None):
    nc = bacc.Bacc()

    xq_d = nc.declare_dram_parameter("xq", [P, DC, S], BF16, isOutput=False)
    xk_d = nc.declare_dram_parameter("xk", [P, DC, S], BF16, isOutput=False)
    xv_d = nc.declare_dram_parameter("xv", [KC, P, DC, P], BF16, isOutput=False)
    wq_d = nc.declare_dram_parameter("wq", [P, DC, HC], BF16, isOutput=False)
    wk_d = nc.declare_dram_parameter("wk", [P, DC, HC], BF16, isOutput=False)
    wv_d = nc.declare_dram_parameter("wv", [P, DC, HC], BF16, isOutput=False)
    wo_d = nc.declare_dram_parameter("wo", [P, LC, D], BF16, isOutput=False)
    bq_d = nc.declare_dram_parameter("bq", [1, HC], BF16, isOutput=False)
    bk_d = nc.declare_dram_parameter("bk", [1, HC], BF16, isOutput=False)
    bv_d = nc.declare_dram_parameter("bv", [1, HC], BF16, isOutput=False)
    out_d = nc.declare_dram_parameter("out", [KC, P, D], F32, isOutput=True)

    with ExitStack() as ctx:
        tc = ctx.enter_context(tile.TileContext(nc))
        pers = ctx.enter_context(tc.tile_pool(name="pers", bufs=1))
        xin = ctx.enter_context(tc.tile_pool(name="xin", bufs=2))
        epool = ctx.enter_context(tc.tile_pool(name="epool", bufs=4))
        rpool = ctx.enter_context(tc.tile_pool(name="rpool", bufs=2))
        pscore = ctx.enter_context(tc.tile_pool(name="pscore", bufs=2, space="PSUM"))
        pctx = ctx.enter_context(tc.tile_pool(name="pctx", bufs=1, space="PSUM"))
        paux = ctx.enter_context(tc.tile_pool(name="paux", bufs=2, space="PSUM"))

        # ---- persistent SBUF tensors ----
        wq = pers.tile([P, DC, HC], BF16, name="wq_t")
        wk = pers.tile([P, DC, HC], BF16, name="wk_t")
        wv = pers.tile([P, DC, HC], BF16, name="wv_t")
        wo = pers.tile([P, LC, D], BF16, name="wo_t")
        bq = pers.tile([1, HC], BF16, name="bq_t")
        bk = pers.tile([1, HC], BF16, name="bk_t")
        bv = pers.tile([1, HC], BF16, name="bv_t")
        ones = pers.tile([1, 512], BF16, name="ones_t")
        qh = pers.tile([P, LC, S], BF16, name="qh_t")     # qhT: [head-col, seq]
        kh = pers.tile([P, LC, S], BF16, name="kh_t")     # khT
        vh = pers.tile([P, KC, NH, HD + 1], BF16, name="vh_t")  # [k-seq, ., head, d|1]
        ctxT = pers.tile([P, LC, S], BF16, name="ctx_t")  # [local-hidden, seq]

        nc.sync.dma_start(wq[:], wq_d[:])
        nc.sync.dma_start(wk[:], wk_d[:])
        nc.sync.dma_start(wv[:], wv_d[:])
        nc.sync.dma_start(wo[:], wo_d[:])
        nc.sync.dma_start(bq[:], bq_d[:])
        nc.sync.dma_start(bk[:], bk_d[:])
        nc.sync.dma_start(bv[:], bv_d[:])
        nc.vector.memset(ones[:], 1.0)
        nc.vector.memset(vh[:, :, :, HD:HD + 1], 1.0)  # PV ones-column

        loop_cm = tc.For_i(0, loop_n, 1) if loop_n else None
        if loop_cm:
            loop_cm.__enter__()
        xq = xin.tile([P, DC, S], BF16, name="xq_t", tag="x")
        for dc in range(DC):
            nc.sync.dma_start(xq[:, dc, :], xq_d[:, dc, :])
        xk = xin.tile([P, DC, S], BF16, name="xk_t", tag="x")
        for dc in range(DC):
            nc.sync.dma_start(xk[:, dc, :], xk_d[:, dc, :])

        def proj_qk(dst, x_t, w_t, b_t, c):
            """dst[:, c, :] = (W_g @ x.T + b) for head-col chunk c, all seq."""
            for sc in range(NSC):
                pt = paux.tile([P, 512], F32, name=f"pp_{c}_{sc}", tag="aux")
                for dc in range(DC):
                    nc.tensor.matmul(
                        pt[:], lhsT=w_t[:, dc, c * P:(c + 1) * P],
                        rhs=x_t[:, dc, sc * 512:(sc + 1) * 512],
                        start=(dc == 0), stop=False)
                nc.tensor.matmul(
                    pt[:], lhsT=b_t[0:1, c * P:(c + 1) * P], rhs=ones[0:1, :],
                    start=False, stop=True)
                nc.vector.tensor_copy(dst[:, c, sc * 512:(sc + 1) * 512], pt[:])

        def proj_v(sc):
            """vh[:, sc, :, 0:64] = (xv @ Wv.T + bv) rows sc*128..+128."""
            xvc = xin.tile([P, DC, P], BF16, name=f"xvc_{sc}", tag="xvc", bufs=4)
            nc.sync.dma_start(xvc[:], xv_d[sc])
            pt = paux.tile([P, 512], F32, name=f"pv_{sc}", tag="aux")
            for dc in range(DC):
                nc.tensor.matmul(
                    pt[:], lhsT=xvc[:, dc, :], rhs=wv[:, dc, :],
                    start=(dc == 0), stop=False)
            nc.tensor.matmul(
                pt[:], lhsT=ones[0:1, 0:P], rhs=bv[0:1, :], start=False, stop=True)
            nc.vector.tensor_copy(
                vh[:, sc, :, 0:HD], pt[:].rearrange("p (h d) -> p h d", h=NH))

        def attn_block(h, qb, kcs):
            """QK + exp + PV for head h, q-block qb, k-chunks kcs."""
            c, po = h // 2, (h % 2) * HD
            q0 = qb * QB
            ctx_t = attn_block.ctx
            for kc in kcs:
                st = pscore.tile([P, QB], F32, name=f"st_{h}_{qb}_{kc}", tag="scores")
                for hf in range(2):
                    nc.tensor.matmul(
                        st[:, hf * 512:(hf + 1) * 512],
                        lhsT=kh[po:po + HD, c, kc * P:(kc + 1) * P],
                        rhs=qh[po:po + HD, c, q0 + hf * 512:q0 + (hf + 1) * 512],
                        start=True, stop=True, tile_position=(po, 0))
                ex = epool.tile([P, QB], BF16, name=f"ex_{h}_{qb}_{kc}", tag="exp", bufs=3)
                nc.scalar.activation(ex[:], st[:], EXP, scale=SCALE)
                for hf in range(2):
                    nc.tensor.matmul(
                        ctx_t[:, hf * 512:(hf + 1) * 512],
                        lhsT=vh[:, kc, h, :],
                        rhs=ex[:, hf * 512:(hf + 1) * 512],
                        start=(kc == 0), stop=(kc == KC - 1))

        def attn_norm(h, qb):
            """Normalize ctx psum by softmax denominators into ctxT sbuf.

            Stage ctx out of PSUM first (2 fast copies) so the psum tile is
            released before the serial recip/broadcast chain runs."""
            c, po = h // 2, (h % 2) * HD
            q0 = qb * QB
            ctx_t = attn_block.ctx
            dn = rpool.tile([1, QB], F32, name=f"dn_{h}_{qb}", tag="den")
            nc.vector.tensor_copy(dn[:], ctx_t[HD:HD + 1, :])
            cu = rpool.tile([HD, QB], BF16, name=f"cu_{h}_{qb}", tag="cu", bufs=2)
            nc.vector.tensor_copy(cu[:], ctx_t[0:HD, :])
            rf = rpool.tile([1, QB], F32, name=f"rf_{h}_{qb}", tag="recf", bufs=1)
            nc.vector.reciprocal_approx_fast(rf[:], dn[:])
            rb = rpool.tile([1, QB], BF16, name=f"rb_{h}_{qb}", tag="recb", bufs=1)
            nc.vector.tensor_copy(rb[:], rf[:])
            for hf in range(2):
                bc = paux.tile([P, 512], F32, name=f"bc_{h}_{qb}_{hf}", tag="aux")
                nc.tensor.matmul(
                    bc[0:HD, :], lhsT=ones[0:1, 0:HD],
                    rhs=rb[0:1, hf * 512:(hf + 1) * 512], start=True, stop=True)
                nc.vector.tensor_mul(
                    ctxT[po:po + HD, c, q0 + hf * 512:q0 + (hf + 1) * 512],
                    cu[:, hf * 512:(hf + 1) * 512], bc[0:HD, :])

        def attn_head(h, weave=None, mid=None):
            for qb in range(2):
                attn_block.ctx = pctx.tile(
                    [HD + 1, QB], F32, name=f"ctx_{h}_{qb}", tag="ctx")
                for k0 in range(0, KC, 4):
                    attn_block(h, qb, range(k0, k0 + 4))
                    if weave:
                        for fn in weave.pop(0) if weave else []:
                            fn()
                attn_norm(h, qb)
                if qb == 0 and mid:
                    mid()

        def outproj(scs):
            for sc in scs:
                for dcol in range(2):
                    ot = paux.tile([P, 512], F32, name=f"op_{sc}_{dcol}", tag="aux")
                    for lc in range(LC):
                        nc.tensor.matmul(
                            ot[:], lhsT=ctxT[:, lc, sc * P:(sc + 1) * P],
                            rhs=wo[:, lc, dcol * 512:(dcol + 1) * 512],
                            start=(lc == 0), stop=(lc == LC - 1))
                    og = epool.tile([P, 512], F32, name=f"og_{sc}_{dcol}", tag="ostage", bufs=2)
                    nc.vector.tensor_copy(og[:], ot[:])
                    nc.sync.dma_start(out_d[sc, :, dcol * 512:(dcol + 1) * 512], og[:])

        # ---- emission order == scheduling priority ----
        proj_qk(qh, xq, wq, bq, 0)
        proj_qk(kh, xk, wk, bk, 0)
        for sc in range(4):
            proj_v(sc)
        # head 0 with just-in-time vh projection woven between kc blocks
        weave = [[lambda s=s: proj_v(s) for s in range(4, 8)],
                 [lambda s=s: proj_v(s) for s in range(8, 12)],
                 [lambda s=s: proj_v(s) for s in range(12, 16)],
                 []]
        attn_head(0, weave=weave)
        attn_head(1)
        for c in range(1, LC):
            proj_qk(qh, xq, wq, bq, c)
            proj_qk(kh, xk, wk, bk, c)
            attn_head(2 * c)
            attn_head(2 * c + 1, mid=(lambda: outproj(range(0, 8))) if c == LC - 1 else None)
        outproj(range(8, KC))
        if loop_cm:
            loop_cm.__exit__(None, None, None)

    nc.finalize()
    return nc


def _get_nc():
    global _NC_CACHE
    if _NC_CACHE is None:
        _NC_CACHE = _build()
    return _NC_CACHE


def _to_dev_x(x):
    # [S, D] f32 -> [P, DC, S] bf16 with [p, dc, s] = x[s, dc*128+p]
    return np.ascontiguousarray(
        x.T.reshape(DC, P, S).transpose(1, 0, 2)).astype(ml_dtypes.bfloat16)


def _to_dev_xv(x):
    # [S, D] f32 -> [KC, P, DC, P] bf16 with [sc, p, dc, s'] = x[sc*128+s', dc*128+p]
    return np.ascontiguousarray(
        x.reshape(KC, P, DC, P).transpose(0, 3, 2, 1)).astype(ml_dtypes.bfloat16)


def _to_dev_w(w, g):
    # Wg = W[g*512:(g+1)*512, :]; dev[p, dc, j] = Wg[j, dc*128+p]
    wg = w[g * HC:(g + 1) * HC, :]
    return np.ascontiguousarray(
        wg.T.reshape(DC, P, HC).transpose(1, 0, 2)).astype(ml_dtypes.bfloat16)


def _to_dev_wo(w, g):
    # Wog = W[:, g*512:(g+1)*512]; dev[p, lc, j] = Wog[j, lc*128+p].T ...
    wog = w[:, g * HC:(g + 1) * HC]  # [D, HC]
    return np.ascontiguousarray(
        wog.T.reshape(LC, P, D).transpose(1, 0, 2)).astype(ml_dtypes.bfloat16)


def kernel(q, k, v, Wq, bq, Wk, bk, Wv, bv, Wo, bo):
    q, k, v = (np.asarray(t, np.float32) for t in (q, k, v))
    Wq, Wk, Wv, Wo = (np.asarray(t, np.float32) for t in (Wq, Wk, Wv, Wo))
    bq, bk, bv, bo = (np.asarray(t, np.float32) for t in (bq, bk, bv, bo))
    nc = _get_nc()

    B = q.shape[0]
    xqs = [_to_dev_x(q[b]) for b in range(B)]
    xks = [_to_dev_x(k[b]) for b in range(B)]
    xvs = [_to_dev_xv(v[b]) for b in range(B)]
    wdev = {}
    for g in range(2):
        wdev[g] = dict(
            wq=_to_dev_w(Wq, g), wk=_to_dev_w(Wk, g), wv=_to_dev_w(Wv, g),
            wo=_to_dev_wo(Wo, g),
            bq=bq[g * HC:(g + 1) * HC].reshape(1, HC).astype(ml_dtypes.bfloat16),
            bk=bk[g * HC:(g + 1) * HC].reshape(1, HC).astype(ml_dtypes.bfloat16),
            bv=bv[g * HC:(g + 1) * HC].reshape(1, HC).astype(ml_dtypes.bfloat16),
        )
    in_maps = []
    for c in range(8):
        b, g = c // 2, c % 2
        in_maps.append(dict(xq=xqs[b], xk=xks[b], xv=xvs[b], **wdev[g]))

    res = run_bass_kernel_spmd(nc, in_maps, core_ids=list(range(8)))

    out = np.empty((B, S, D), np.float32)
    for b in range(B):
        p0 = res.results[2 * b]["out"].reshape(S, D)
        p1 = res.results[2 * b + 1]["out"].reshape(S, D)
        out[b] = p0 + p1 + bo
    return out
